# revision 1
# baseline (speedup 1.0000x reference)
"""3x3 SAME conv + ReLU on 8 TRN2 cores — hand-semaphored PE pipeline.

Same implicit-GEMM mapping as before (spatial H-shard, channel-major input
via xbar transpose DMA, pixel-major PSUM accumulation over 9 taps), but the
whole compute pipeline (450 matmuls, DVE ReLUs, stores) runs inside one
tc.tile_critical() region with three hand-managed semaphores and 8 rotating
raw PSUM banks / SBUF out-buffers. This removes Tile's per-matmul semaphore
update (~44 ns/matmul measured), taking the PE stream from ~174 to ~130
ns/matmul.
"""

import sys
from contextlib import ExitStack

sys.path.insert(0, "/opt/trn_rl_repo")

import ml_dtypes
import numpy as np

H = 224
WID = 224
C_IN = 128
C_OUT = 256
KK = 3
NCORES = 8
RPC = H // NCORES
WP = WID + 2
HALO = RPC + 2
NPIX = HALO * WP
T_TILES = 50
YROWS = T_TILES * 128
XROWS = 6864
N_WARM = 26
NBANK = 8

COMPUTE_DT = ml_dtypes.bfloat16

_COMPILED = None
LAST_RESULT = None


def _build():
    import concourse.bacc as bacc
    import concourse.mybir as mybir
    import concourse.tile as tile

    nc = bacc.Bacc("TRN2", target_bir_lowering=False, debug=False, num_devices=NCORES)
    MDT = mybir.dt.from_np(np.dtype(COMPUTE_DT))

    x_d = nc.dram_tensor("xs", [XROWS, C_IN], MDT, kind="ExternalInput").ap()
    w_d = nc.dram_tensor("w", [C_IN, 9 * C_OUT], MDT, kind="ExternalInput").ap()
    y_d = nc.dram_tensor("y", [YROWS, C_OUT], MDT, kind="ExternalOutput").ap()

    with ExitStack() as stack:
        banks = [
            stack.enter_context(
                nc.psum_tensor(f"bank{i}", [128, C_OUT], mybir.dt.float32)
            )
            for i in range(NBANK)
        ]
        ots = [
            stack.enter_context(nc.sbuf_tensor(f"ot{i}", [128, C_OUT], MDT))
            for i in range(NBANK)
        ]
        mm_sem = stack.enter_context(nc.semaphore("mm_sem"))
        act_sem = stack.enter_context(nc.semaphore("act_sem"))
        st_sems = [
            stack.enter_context(nc.semaphore(f"st_sem{i}")) for i in range(NBANK)
        ]

        with tile.TileContext(nc) as tc:
            with (
                tc.tile_pool(name="xt", bufs=1) as xt_pool,
                tc.tile_pool(name="wt", bufs=1) as wt_pool,
            ):
                wt = wt_pool.tile([C_IN, 9 * C_OUT], MDT)
                nc.sync.dma_start(wt[:], w_d[:])
                xt = xt_pool.tile([C_IN, XROWS], MDT)
                nc.sync.dma_start(xt[:], x_d[:], transpose=True)

                # PE clock-gate warmup on the weight tile while input loads
                # (writes bank 7; first reused by group 7 with start=True).
                for i in range(N_WARM):
                    nc.tensor.matmul(
                        banks[7].ap(), wt[:, 0:128], wt[:, 0:C_OUT],
                        start=(i == 0), stop=(i == N_WARM - 1),
                        skip_group_check=True,
                    )

                with tc.tile_critical():
                    for t in range(T_TILES):
                        b = t % NBANK
                        if t >= NBANK:
                            nc.tensor.wait_ge(act_sem, t - NBANK + 1)
                        for kh in range(KK):
                            for kw in range(KK):
                                k = kh * KK + kw
                                off = t * 128 + kh * WP + kw
                                mm = nc.tensor.matmul(
                                    banks[b].ap(),
                                    xt[:, off : off + 128],
                                    wt[:, k * C_OUT : (k + 1) * C_OUT],
                                    start=(k == 0),
                                    stop=(k == 8),
                                    skip_group_check=True,
                                )
                                if k == 8:
                                    mm.then_inc(mm_sem, 1)
                        nc.vector.wait_ge(mm_sem, t + 1)
                        if t >= NBANK:
                            nc.vector.wait_ge(st_sems[b], 16 * (t // NBANK))
                        nc.vector.tensor_scalar_max(
                            ots[b].ap(), banks[b].ap(), 0.0
                        ).then_inc(act_sem, 1)
                        nc.sync.wait_ge(act_sem, t + 1)
                        nc.sync.dma_start(
                            y_d[t * 128 : (t + 1) * 128, :], ots[b].ap()
                        ).then_inc(st_sems[b], 16)

        nc.compile()
    return nc


def _prep_inputs(x: np.ndarray, W: np.ndarray):
    xp = np.zeros((H + 2, WP, C_IN), np.float32)
    xp[1 : H + 1, 1 : WID + 1] = x
    xs = np.zeros((NCORES, XROWS, C_IN), COMPUTE_DT)
    for i in range(NCORES):
        xs[i, 1 : 1 + NPIX] = (
            xp[RPC * i : RPC * i + HALO].reshape(NPIX, C_IN).astype(COMPUTE_DT)
        )
    wh = (
        W.reshape(C_OUT, 9, C_IN)
        .transpose(2, 1, 0)
        .reshape(C_IN, 9 * C_OUT)
        .astype(COMPUTE_DT)
    )
    return xs, wh


def kernel(x: np.ndarray, W: np.ndarray) -> np.ndarray:
    global _COMPILED, LAST_RESULT
    from concourse import bass_utils

    if _COMPILED is None:
        _COMPILED = _build()
    nc = _COMPILED

    xs, wh = _prep_inputs(np.asarray(x, np.float32), np.asarray(W, np.float32))
    in_maps = [{"xs": np.ascontiguousarray(xs[i]), "w": wh} for i in range(NCORES)]

    try:
        res = bass_utils.run_bass_kernel_spmd(nc, in_maps, core_ids=list(range(NCORES)))
    except Exception:
        import os

        if os.environ.get("BASS_TRACE"):
            os.environ.pop("BASS_TRACE", None)
            res = bass_utils.run_bass_kernel_spmd(
                nc, in_maps, core_ids=list(range(NCORES))
            )
        else:
            raise
    LAST_RESULT = res

    y = np.stack([r["y"] for r in res.results])
    y = y[:, : RPC * WP].reshape(NCORES, RPC, WP, C_OUT)[:, :, 1 : WID + 1]
    return y.reshape(H, WID, C_OUT).astype(np.float32)

